# revision 12
# baseline (speedup 1.0000x reference)
"""TreeLSTM cell (binary children) on 8 Trainium2 NeuronCores.

Data-parallel over the node batch B=262144: each core processes 32768 rows.
All device I/O is float16 (tolerance is 2e-2; fp16 end-to-end error is
~5e-4), which halves HBM traffic vs f32: 7 x 8.39MB = 58.7MB per core,
~165us at the ~360GB/s per-core DMA roofline.

Host-side data prep (not part of device time):
  - inputs/l_h/r_h are transposed to [128, rows] and packed into one
    [128, 3*B_CORE] fp16 tensor (contraction dim on partitions).
  - l_c/r_c (and the h/c outputs) are stored partition-major per chunk:
    npk[p, s*B_CORE + c0 + j*128 + f] = (l_c|r_c)[c0 + j*128 + p, f],
    so every DMA is 2-3 contiguous 4-6KB runs per partition.

Per 128-row block (batch rows on PSUM partitions, features on free axis):
  A bank   = [i|o|fl|fr] = x@[Wxi|Wxo|Wfx|Wfx] + hsum@[Whi|Who|0|0]
             + lh@[0|0|Wfh|0] + rh@[0|0|0|Wfh]      (hsum = lh+rh on DVE)
  U region = x@Wxu + hsum@Whu                        (u pre-activation)
Supers of 3 blocks share one [128,3,512] A tile (3 banks) + one [128,384]
U tile; with bufs=2 that is exactly the 8 PSUM banks. One sigmoid ACT
instruction covers all 4 gates x 3 blocks (free=1536); tanh(u) is one more
small one; tanh(c) is one chunk-wide instruction. The c/h elementwise
chain runs on DVE as tensor_tensor ops (2x fp16 mode; scalar_tensor_tensor
has no DVE perf modes and runs 1x) with fr*rc on the Pool engine. The last
chunk's output DMA is issued from the ACT queue so SP's wraparound to the
next loop iteration's input prefetch is never blocked. Modeled engine busy
per core: ACT 200us (bottleneck: 6 activation passes at 1 elem/lane/cycle
@1.2GHz = 164us floor), DMA 164us, PE 140us, DVE 129us.
"""

import numpy as np

import concourse.bass as bass
import concourse.tile as tile
from concourse import mybir

FP = mybir.dt.float32
F16 = mybir.dt.float16
N_CORES = 8
B = 262144
D = 128
B_CORE = B // N_CORES          # 32768
# chunks of 3072 rows = 24 blocks = 8 supers of 3; tail chunk of 2048 rows
CHUNKS = [(i * 3072, 3072) for i in range(10)] + [(30720, 2048)]
W_COLS = 1152                  # [WA(512)|WB(256)|WFH(128)|WXU(128)|WHU(128)]

Sig = mybir.ActivationFunctionType.Sigmoid
Tanh = mybir.ActivationFunctionType.Tanh
MUL = mybir.AluOpType.mult
ADD = mybir.AluOpType.add

LAST_RESULT = None
_PROGRAM_CACHE = {}


def _split_multi_waits(nc):
    """This walrus codegen allows only one semaphore wait per instruction;
    Tile's scheduler freely attaches several. Hoist the extras onto fresh
    same-engine NoOps placed immediately before the instruction."""
    for blk in nc.main_func.blocks:
        new_insts = []
        for inst in blk.instructions:
            si = inst.sync_info
            if si is not None and si.on_wait and len(si.on_wait) > 1:
                waits = list(si.on_wait)
                upd = list(si.on_update) if si.on_update else []
                for w in waits[:-1]:
                    nop = mybir.InstNoOp(
                        name=nc.get_next_instruction_name(), ins=[], outs=[])
                    nop.engine = inst.engine
                    nop.sync_info = mybir.SyncInfo(on_wait=[w], on_update=[])
                    nc.register_instruction(nop)
                    new_insts.append(nop)
                inst.sync_info = mybir.SyncInfo(
                    on_wait=[waits[-1]], on_update=upd)
            new_insts.append(inst)
        blk.instructions[:] = new_insts


def _supers():
    """Global list of (chunk_idx, s_in_chunk, block0, ns)."""
    out = []
    for ci, (_, ch) in enumerate(CHUNKS):
        nblk = ch // 128
        s = 0
        b0 = 0
        while b0 < nblk:
            ns = min(3, nblk - b0)
            out.append((ci, s, b0, ns))
            s += 1
            b0 += ns
    return out


def _build_program(with_bias: bool, bench_loops: int = 0):
    nc = bass.Bass()
    tpk = nc.dram_tensor("tpk", [D, 3 * B_CORE], F16, kind="ExternalInput")
    npk = nc.dram_tensor("npk", [D, 2 * B_CORE], F16, kind="ExternalInput")
    wt = nc.dram_tensor("wt", [D, W_COLS], F16, kind="ExternalInput")
    if with_bias:
        biasA = nc.dram_tensor("biasA", [1, 512], F16, kind="ExternalInput")
        biasU = nc.dram_tensor("biasU", [1, 128], F16, kind="ExternalInput")
        ones = nc.dram_tensor("ones", [1, 128], F16, kind="ExternalInput")
    hc = nc.dram_tensor("hc", [D, 2 * B_CORE], F16, kind="ExternalOutput")

    tpk_v = tpk[:].rearrange("p (s n) -> p s n", s=3)
    npk_v = npk[:].rearrange("p (s n) -> p s n", s=2)
    hc_v = hc[:].rearrange("p (s n) -> p s n", s=2)

    supers = _supers()
    S = len(supers)
    CH_MAX = 3072
    NB_MAX = CH_MAX // 128

    with tile.TileContext(nc) as tc:
        with (
            tc.tile_pool(name="w", bufs=1) as wpool,
            tc.tile_pool(name="ins", bufs=2) as inpool,
            tc.tile_pool(name="outs", bufs=2) as outpool,
            tc.tile_pool(name="gates", bufs=2) as gpool,
            tc.tile_pool(name="gu", bufs=3) as gupool,
            tc.tile_pool(name="hs", bufs=4) as hspool,
            tc.tile_pool(name="tmp", bufs=3) as tpool,
            tc.tile_pool(name="tc", bufs=2) as tcpool,
            tc.tile_pool(name="psA", bufs=2, space=bass.MemorySpace.PSUM) as apool,
            tc.tile_pool(name="psU", bufs=2, space=bass.MemorySpace.PSUM) as upool,
        ):
            w_t = wpool.tile([D, W_COLS], F16)
            nc.sync.dma_start(w_t[:], wt[:])
            WA = w_t[:, 0:512]
            WB = w_t[:, 512:768]
            WFH = w_t[:, 768:896]
            WXU = w_t[:, 896:1024]
            WHU = w_t[:, 1024:1152]
            bA = bU = one_t = None
            if with_bias:
                bA = wpool.tile([1, 512], F16)
                nc.sync.dma_start(bA[:], biasA[:])
                bU = wpool.tile([1, 128], F16)
                nc.sync.dma_start(bU[:], biasU[:])
                one_t = wpool.tile([1, 128], F16)
                nc.sync.dma_start(one_t[:], ones[:])

            def emit_all():
                # per-chunk live tiles, keyed by chunk idx
                tp_t, np_t, g_t, out_t, tc_t = {}, {}, {}, {}, {}
                hs_t = {}  # per-super hsum tiles

                def dma_in(ci):
                    c0, ch = CHUNKS[ci]
                    tp = inpool.tile([D, 3, CH_MAX], F16, tag="tp", name="tp")
                    nc.sync.dma_start(tp[:, :, 0:ch], tpk_v[:, :, c0:c0 + ch])
                    np_ = inpool.tile([D, 2, CH_MAX], F16, tag="np", name="npt")
                    nc.sync.dma_start(np_[:, :, 0:ch], npk_v[:, :, c0:c0 + ch])
                    tp_t[ci] = tp
                    np_t[ci] = np_

                def emit_hsum(gs):
                    ci, _, b0, ns = supers[gs]
                    tp = tp_t[ci]
                    cols = slice(128 * b0, 128 * (b0 + ns))
                    hs = hspool.tile([D, 384], F16, tag="hs", name="hs")
                    nc.vector.tensor_add(
                        hs[:, 0:128 * ns], tp[:, 1, cols], tp[:, 2, cols])
                    hs_t[gs] = hs

                def finish_chunk(ci, last=False):
                    # second tanh(c) half, h = o*tanh(c), output DMA
                    _, ch = CHUNKS[ci]
                    nblk = ch // 128
                    out = out_t[ci]
                    tct = tc_t[ci]
                    nc.scalar.activation(tct[:, 0:ch], out[:, 1, 0:ch], Tanh)
                    G = g_t[ci]
                    o_v = G[:, 0:nblk, 128:256]
                    tc_v = tct[:, 0:ch].rearrange("p (j f) -> p j f", f=128)
                    h_v = out[:, 0, 0:ch].rearrange("p (j f) -> p j f", f=128)
                    nc.vector.tensor_mul(h_v, o_v, tc_v)
                    c0, ch = CHUNKS[ci]
                    # The last chunk's output DMA would head-of-line
                    # block SP's wraparound to the next loop iteration's
                    # input prefetch (it waits on h of the final chunk), so
                    # issue it from the ACT queue instead; mid-stream
                    # outputs stay on SP where their waits are satisfied
                    # long before the next input issue.
                    if last:
                        nc.scalar.dma_start(hc_v[:, :, c0:c0 + ch],
                                            out[:, :, 0:ch])
                    else:
                        nc.sync.dma_start(hc_v[:, :, c0:c0 + ch],
                                          out[:, :, 0:ch])

                for gs in range(S):
                    ci, s, b0, ns = supers[gs]
                    c0, ch = CHUNKS[ci]
                    if gs == 0:
                        dma_in(0)
                        emit_hsum(0)
                        emit_hsum(1)
                    if s == 0:
                        g_t[ci] = gpool.tile([D, NB_MAX, 512], F16, tag="G", name="G")
                        out_t[ci] = outpool.tile([D, 2, CH_MAX], F16, tag="out", name="out")
                        tc_t[ci] = tcpool.tile([D, CH_MAX], F16, tag="tc", name="tct")
                    if s == 4 and ci + 1 < len(CHUNKS):
                        dma_in(ci + 1)

                    tp = tp_t[ci]
                    hs = hs_t.pop(gs)
                    A = apool.tile([D, 3, 512], FP, tag="A", name="A")
                    U = upool.tile([D, 384], FP, tag="U", name="U")

                    # matmuls: ordered so each 128-row stationary load hides
                    # under the previous matmul's moving stream
                    for k in range(ns):
                        j = b0 + k
                        xb = tp[:, 0, j * 128:(j + 1) * 128]
                        lb = tp[:, 1, j * 128:(j + 1) * 128]
                        rb = tp[:, 2, j * 128:(j + 1) * 128]
                        hb = hs[:, k * 128:(k + 1) * 128]
                        Ak = A[:, k, :]
                        Uk = U[:, k * 128:(k + 1) * 128]
                        if with_bias:
                            nc.tensor.matmul(Ak, one_t[:], bA[:],
                                             start=True, stop=False,
                                             skip_group_check=True)
                            nc.tensor.matmul(Uk, one_t[:], bU[:],
                                             start=(k == 0), stop=False,
                                             skip_group_check=True)
                        nc.tensor.matmul(Ak, xb, WA,
                                         start=not with_bias, stop=False,
                                         skip_group_check=True)
                        nc.tensor.matmul(Uk, xb, WXU,
                                         start=(k == 0 and not with_bias),
                                         stop=False, skip_group_check=True)
                        nc.tensor.matmul(Ak[:, 0:256], hb, WB,
                                         start=False, stop=False,
                                         skip_group_check=True)
                        nc.tensor.matmul(Uk, hb, WHU,
                                         start=False, stop=(k == ns - 1),
                                         skip_group_check=True)
                        nc.tensor.matmul(Ak[:, 256:384], lb, WFH,
                                         start=False, stop=False,
                                         skip_group_check=True)
                        nc.tensor.matmul(Ak[:, 384:512], rb, WFH,
                                         start=False, stop=True,
                                         skip_group_check=True)

                    # activations
                    G = g_t[ci]
                    gu = gupool.tile([D, 384], F16, tag="gu", name="gu")
                    nc.scalar.activation(G[:, b0:b0 + ns, :], A[:, 0:ns, :],
                                         Sig)
                    nc.scalar.activation(gu[:, 0:128 * ns], U[:, 0:128 * ns],
                                         Tanh)
                    # hsum lookahead so PE(s+2) never waits on the DVE queue
                    if gs + 2 < S:
                        emit_hsum(gs + 2)
                    # previous chunk tail work, now that ACT/DVE moved on
                    if s == 0 and ci > 0:
                        finish_chunk(ci - 1)

                    # elementwise c chain (fp16 4x stt ops)
                    cols = slice(128 * b0, 128 * (b0 + ns))
                    i_v = G[:, b0:b0 + ns, 0:128]
                    fl_v = G[:, b0:b0 + ns, 256:384]
                    fr_v = G[:, b0:b0 + ns, 384:512]
                    u_v = gu[:, 0:128 * ns].rearrange("p (k f) -> p k f", f=128)
                    lc_v = np_t[ci][:, 0, cols].rearrange(
                        "p (k f) -> p k f", f=128)
                    rc_v = np_t[ci][:, 1, cols].rearrange(
                        "p (k f) -> p k f", f=128)
                    c_v = out_t[ci][:, 1, cols].rearrange(
                        "p (k f) -> p k f", f=128)
                    T = tpool.tile([D, 3, 3, 128], F16, tag="T", name="T")
                    t1 = T[:, 0, 0:ns, :]
                    t2 = T[:, 1, 0:ns, :]
                    t3 = T[:, 2, 0:ns, :]
                    nc.vector.tensor_mul(t1, i_v, u_v)
                    nc.gpsimd.tensor_mul(t3, fr_v, rc_v)
                    nc.vector.tensor_mul(t2, fl_v, lc_v)
                    nc.vector.tensor_add(t2, t1, t2)
                    nc.vector.tensor_add(c_v, t2, t3)

                finish_chunk(len(CHUNKS) - 1, last=True)

            if bench_loops:
                with tc.For_i(0, bench_loops, 1):
                    emit_all()
            else:
                emit_all()

    _split_multi_waits(nc)
    return nc


def _get_program(with_bias: bool):
    if with_bias not in _PROGRAM_CACHE:
        _PROGRAM_CACHE[with_bias] = _build_program(with_bias)
    return _PROGRAM_CACHE[with_bias]


class _Runner:
    """Compiled 8-core SPMD executable for one Bass program (the jit body
    mirrors concourse.bass2jax.run_bass_via_pjrt, but is built once and
    reused so repeat kernel() calls and benchmarking skip recompilation)."""

    def __init__(self, nc):
        import jax
        from jax.sharding import Mesh, PartitionSpec, NamedSharding
        from jax.experimental.shard_map import shard_map
        from concourse import bass2jax

        bass2jax.install_neuronx_cc_hook()
        self.jax = jax
        part_name = nc.partition_id_tensor.name if nc.partition_id_tensor else None
        in_names, out_names, out_avals, zero_outs = [], [], [], []
        for alloc in nc.m.functions[0].allocations:
            if not isinstance(alloc, mybir.MemoryLocationSet):
                continue
            name = alloc.memorylocations[0].name
            if alloc.kind == "ExternalInput":
                if name != part_name:
                    in_names.append(name)
            elif alloc.kind == "ExternalOutput":
                out_names.append(name)
                shape = tuple(alloc.tensor_shape)
                dtype = mybir.dt.np(alloc.dtype)
                out_avals.append(jax.core.ShapedArray(shape, dtype))
                zero_outs.append(np.zeros(shape, dtype))
        self.in_names = list(in_names)
        self.out_names = out_names
        self.out_avals = out_avals
        self.zero_outs = zero_outs
        n_params = len(in_names)
        all_in_names = in_names + out_names
        if part_name is not None:
            all_in_names = all_in_names + [part_name]

        def _body(*args):
            operands = list(args)
            if part_name is not None:
                operands.append(bass2jax.partition_id_tensor())
            outs = bass2jax._bass_exec_p.bind(
                *operands,
                out_avals=tuple(out_avals),
                in_names=tuple(all_in_names),
                out_names=tuple(out_names),
                lowering_input_output_aliases=(),
                sim_require_finite=True,
                sim_require_nnan=True,
                nc=nc,
            )
            return tuple(outs)

        devices = jax.devices()[:N_CORES]
        self.mesh = Mesh(np.asarray(devices), ("core",))
        self.sharding = NamedSharding(self.mesh, PartitionSpec("core"))
        in_specs = (PartitionSpec("core"),) * (n_params + len(out_names))
        out_specs = (PartitionSpec("core"),) * len(out_names)
        self.fn = jax.jit(
            shard_map(_body, mesh=self.mesh, in_specs=in_specs,
                      out_specs=out_specs, check_rep=False),
            keep_unused=True,
        )

    def stage(self, in_maps):
        """device_put concatenated inputs (+ zero output buffers) once."""
        jax = self.jax
        concat = [
            np.concatenate([m[name] for m in in_maps], axis=0)
            for name in self.in_names
        ]
        concat += [
            np.zeros((N_CORES * z.shape[0], *z.shape[1:]), z.dtype)
            for z in self.zero_outs
        ]
        return [jax.device_put(a, self.sharding) for a in concat]

    def run(self, staged):
        outs = self.fn(*staged)
        self.jax.block_until_ready(outs)
        return outs

    def results(self, outs):
        per_core = []
        for c in range(N_CORES):
            d = {}
            for i, name in enumerate(self.out_names):
                d[name] = np.asarray(outs[i]).reshape(
                    N_CORES, *self.out_avals[i].shape)[c]
            per_core.append(d)
        return per_core


def _get_runner(with_bias: bool):
    key = ("runner", with_bias)
    if key not in _PROGRAM_CACHE:
        _PROGRAM_CACHE[key] = _Runner(_get_program(with_bias))
    return _PROGRAM_CACHE[key]


def _perm(a16):
    """[B_CORE, 128] -> [128, B_CORE] partition-major per chunk:
    out[p, c0 + j*128 + f] = a[c0 + j*128 + p, f]."""
    out = np.empty((D, B_CORE), dtype=np.float16)
    for c0, ch in CHUNKS:
        nblk = ch // 128
        out[:, c0:c0 + ch] = (
            a16[c0:c0 + ch].reshape(nblk, D, D).transpose(1, 0, 2)
            .reshape(D, ch))
    return out


def _unperm(a16):
    out = np.empty((B_CORE, D), dtype=np.float32)
    for c0, ch in CHUNKS:
        nblk = ch // 128
        out[c0:c0 + ch] = (
            a16[:, c0:c0 + ch].reshape(D, nblk, D).transpose(1, 0, 2)
            .reshape(ch, D).astype(np.float32))
    return out


def kernel(l_h, l_c, r_h, r_c, inputs, W_ioux, b_ioux, W_iouh, b_iouh,
           W_fx, b_fx, W_fh, b_fh):
    global LAST_RESULT
    f16 = lambda a: np.asarray(a, dtype=np.float16)
    l_h, l_c, r_h, r_c, inputs = map(f16, (l_h, l_c, r_h, r_c, inputs))
    W_ioux, W_iouh, W_fx, W_fh = map(f16, (W_ioux, W_iouh, W_fx, W_fh))
    b_ioux = np.asarray(b_ioux, dtype=np.float32)
    b_iouh = np.asarray(b_iouh, dtype=np.float32)
    b_fx = np.asarray(b_fx, dtype=np.float32)
    b_fh = np.asarray(b_fh, dtype=np.float32)

    with_bias = bool(np.any(b_ioux) or np.any(b_iouh)
                     or np.any(b_fx) or np.any(b_fh))

    # [WA = Wxi|Wxo|Wfx|Wfx (512)] [WB = Whi|Who (256)] [WFH] [WXU] [WHU]
    wt_host = np.concatenate([
        W_ioux[:, 0:256], W_fx, W_fx,
        W_iouh[:, 0:256], W_fh,
        W_ioux[:, 256:384], W_iouh[:, 256:384]], axis=1)
    wt_host = np.ascontiguousarray(wt_host, dtype=np.float16)
    if with_bias:
        bf = (b_fx + b_fh).reshape(1, 128)
        biasA = np.concatenate(
            [(b_ioux[:256] + b_iouh[:256]).reshape(1, 256), bf, bf],
            axis=1).astype(np.float16)
        biasU = (b_ioux[256:] + b_iouh[256:]).reshape(1, 128).astype(np.float16)
        ones = np.ones((1, 128), dtype=np.float16)

    in_maps = []
    for core in range(N_CORES):
        sl = slice(core * B_CORE, (core + 1) * B_CORE)
        m = {
            "tpk": np.ascontiguousarray(np.concatenate(
                [inputs[sl].T, l_h[sl].T, r_h[sl].T], axis=1)),
            "npk": np.ascontiguousarray(np.concatenate(
                [_perm(l_c[sl]), _perm(r_c[sl])], axis=1)),
            "wt": wt_host,
        }
        if with_bias:
            m["biasA"] = biasA
            m["biasU"] = biasU
            m["ones"] = ones
        in_maps.append(m)

    runner = _get_runner(with_bias)
    staged = runner.stage(in_maps)
    outs = runner.run(staged)
    per_core = runner.results(outs)
    LAST_RESULT = (runner, staged)
    h = np.concatenate(
        [_unperm(d["hc"][:, :B_CORE]) for d in per_core], axis=0)
    c = np.concatenate(
        [_unperm(d["hc"][:, B_CORE:]) for d in per_core], axis=0)
    return h, c
